# revision 11
# baseline (speedup 1.0000x reference)
"""Sparse-attention Trainium2 kernel (nn_AttentionLayer, B=16 S=2048 D=128).

reference semantics:
    A = Q @ T^T                     # [B,S,S]
    A = where(A > 0.3, A, 0)
    A += where(strictly_upper, -2^32, 0)
    y = softmax(A / sqrt(D)) @ V

Sharding: data-parallel over batch, 2 batches per core on 8 NeuronCores.

Per-core algorithm (per batch), v3:
  - Q, T cast to bf16 on VectorE into one combined staging tile, then
    Q^T/T^T built with a few large DMA xbar transposes (zero PE time).
    Prep is chunked so the first q-block can start ~4us after the
    preamble; batch 1's prep is interleaved into batch 0's main loop.
  - Scores computed transposed, S^T[k,q], 2 ktiles per [128,1024] PSUM
    tile; straddling-diagonal k-tiles use a trimmed rhs (dead query
    columns skipped).
  - num = max(exp(S^T*scale),1) via ScalarE exp + VectorE max; for
    straddling tiles the max is fused with the causal mask in one
    scalar_tensor_tensor: num = (exp max 1) * tri_pad01, where
    tri_pad01 is 0 above the in-tile diagonal and 1 elsewhere.
  - PV + denominator fused per (ktile, q-subtile): lhsT = num chunk,
    rhs = [V | ones] [128k,129], accumulated in PSUM; obanks packed two
    per PSUM bank at 1KB offsets. Dead chunks skipped outright.
  - out = PV/den via a VectorE PSUM->SBUF copy + GpSimd normalize_recip.
  - PE warm-up matmuls at t=0 ramp the p-state through the DMA prep.
  - Software pipelining: PV for group g is emitted two QK groups later.
"""

from collections import deque
from contextlib import ExitStack

import numpy as np

import concourse.bass as bass
import concourse.mybir as mybir
import concourse.tile as tile
from concourse import bacc

B, S, D = 16, 2048, 128
N_CORES = 8
B_LOC = B // N_CORES
QB = 512
KT = 128
N_QB = S // QB                # 4 q-blocks
N_ST = S // 128               # 16 seq tiles
SCALE = float(1.0 / np.sqrt(D))

F32 = mybir.dt.float32
BF16 = mybir.dt.bfloat16
Alu = mybir.AluOpType


def build_attention_core():
    nc = bacc.Bacc("TRN2", target_bir_lowering=False, debug=False,
                   num_devices=N_CORES)
    q_ext = nc.dram_tensor("Q", [B_LOC, S, D], F32, kind="ExternalInput").ap()
    t_ext = nc.dram_tensor("T", [B_LOC, S, D], F32, kind="ExternalInput").ap()
    v_ext = nc.dram_tensor("V", [B_LOC, S, D], F32, kind="ExternalInput").ap()
    o_ext = nc.dram_tensor("out", [B_LOC, S, D], F32, kind="ExternalOutput").ap()

    with tile.TileContext(nc) as tc, ExitStack() as ctx:
        const_pool = ctx.enter_context(tc.tile_pool(name="const", bufs=1))
        nat_pool = ctx.enter_context(tc.tile_pool(name="nat", bufs=1))
        stage_pool = ctx.enter_context(tc.tile_pool(name="stage", bufs=1))
        tpd_pool = ctx.enter_context(tc.tile_pool(name="tpd", bufs=1))
        vb_pool = ctx.enter_context(tc.tile_pool(name="vb", bufs=1))
        num_pool = ctx.enter_context(tc.tile_pool(name="num", bufs=4))
        fin_pool = ctx.enter_context(tc.tile_pool(name="fin", bufs=2))
        rec_pool = ctx.enter_context(tc.tile_pool(name="rec", bufs=4))
        qk_psum = ctx.enter_context(tc.tile_pool(name="qk_ps", bufs=2, space="PSUM"))
        ob_psum = ctx.enter_context(tc.tile_pool(name="ob_ps", bufs=4, space="PSUM"))

        # ---- constants (gpsimd) ----
        # junk first: it gates the PE warm-up
        junk = const_pool.tile([128, 512], BF16, name="junk")
        nc.gpsimd.memset(junk[:], 0.25)
        # tri_pad01[p, n] = 0 if p > n else 1  (ones for n >= 128 too)
        tri_pad01 = const_pool.tile([128, 512], BF16, name="tri_pad01")
        nc.gpsimd.memset(tri_pad01[:], 1.0)
        nc.gpsimd.affine_select(
            out=tri_pad01[:], in_=tri_pad01[:],
            compare_op=Alu.is_ge, fill=0.0,
            base=0, channel_multiplier=-1, pattern=[[1, 512]])

        # ---- PE warm-up: ramp the p-state while DMA prep runs ----
        for w in range(12):
            wps = qk_psum.tile([128, 1024], F32, tag="qk", name=f"wps{w}")
            nc.tensor.matmul(wps[:, 0:512], lhsT=junk[:, 0:128], rhs=junk[:])

        # ---- prep plumbing ----
        # combined bf16 staging layout per batch: [q0:4 | t0:4 | q4:16 | t4:16]
        # (each span contiguous so one dma_start_transpose covers it)
        qt_bfs, qt_tps, v_augs = [], [], []
        nats = []
        for b in range(B_LOC):
            q_nat = nat_pool.tile([128, N_ST, D], F32, name=f"qnat{b}")
            t_nat = nat_pool.tile([128, N_ST, D], F32, name=f"tnat{b}")
            v_nat = nat_pool.tile([128, N_ST, D], F32, name=f"vnat{b}")
            qt_bf = stage_pool.tile([128, 2 * N_ST, 128], BF16, name=f"qtbf{b}")
            qt_tp = tpd_pool.tile([128, 2 * N_ST, 128], BF16, name=f"qttp{b}")
            v_aug = vb_pool.tile([128, N_ST, 129], BF16, name=f"vaug{b}")
            nats.append((q_nat, t_nat, v_nat))
            qt_bfs.append(qt_bf); qt_tps.append(qt_tp); v_augs.append(v_aug)

        # staged slot index of transposed tiles:
        #   q tile t (0..15): slot t if t < 4 else 8 + (t - 4)
        #   t tile c (0..15): slot 4 + c if c < 4 else 20 + (c - 4)
        def q_slot(t):
            return t if t < 4 else 8 + (t - 4)

        def t_slot(c):
            return 4 + c if c < 4 else 20 + (c - 4)

        def load_qt(b, h):
            """Issue DMA loads for q/t chunk h (h=0: tiles 0:4, h=1: 4:16)."""
            q_nat, t_nat, v_nat = nats[b]
            sl = slice(0, 4) if h == 0 else slice(4, 16)
            ssl = slice(0, 512) if h == 0 else slice(512, 2048)
            nc.sync.dma_start(
                q_nat[:, sl, :],
                q_ext[b, ssl, :].rearrange("(t p) d -> p t d", p=128))
            nc.sync.dma_start(
                t_nat[:, sl, :],
                t_ext[b, ssl, :].rearrange("(t p) d -> p t d", p=128))

        def load_v(b):
            q_nat, t_nat, v_nat = nats[b]
            nc.sync.dma_start(
                v_nat[:], v_ext[b].rearrange("(t p) d -> p t d", p=128))

        def cast_qt(b, h):
            q_nat, t_nat, v_nat = nats[b]
            qt_bf = qt_bfs[b]
            if h == 0:
                nc.vector.tensor_copy(qt_bf[:, 0:4, :], q_nat[:, 0:4, :])
                nc.vector.tensor_copy(qt_bf[:, 4:8, :], t_nat[:, 0:4, :])
            else:
                nc.vector.tensor_copy(qt_bf[:, 8:20, :], q_nat[:, 4:16, :])
                nc.vector.tensor_copy(qt_bf[:, 20:32, :], t_nat[:, 4:16, :])

        def cast_v(b, h):
            q_nat, t_nat, v_nat = nats[b]
            sl = slice(0, 4) if h == 0 else slice(4, 16)
            nc.gpsimd.memset(v_augs[b][:, sl, D:D + 1], 1.0)
            nc.vector.tensor_copy(v_augs[b][:, sl, 0:D], v_nat[:, sl, :])

        def transpose_span(b, lo_t, hi_t):
            """One xbar transpose over staged slots [lo_t, hi_t)."""
            qt_bf, qt_tp = qt_bfs[b], qt_tps[b]
            nc.sync.dma_start_transpose(
                qt_tp[:, lo_t:hi_t, :],
                qt_bf[:, lo_t:hi_t, :].rearrange("p t d -> p (t d)"))

        # ---- batch-0 head: quarter chunk first, then the rest ----
        load_qt(0, 0)
        load_qt(0, 1)
        load_v(0)
        cast_qt(0, 0)
        transpose_span(0, 0, 8)
        cast_v(0, 0)

        # ---- main loop over (batch, q-block, ktile-pair group) ----
        items = []
        for b in range(B_LOC):
            for qb in range(N_QB):
                for g in range((4 * qb + 4) // 2):
                    items.append((b, qb, g))

        # deferred prep actions keyed by item index
        prep_at = {
            0: lambda: cast_qt(0, 1),
            1: lambda: transpose_span(0, 8, 20),      # q tiles 4..15
            2: lambda: transpose_span(0, 20, 32),     # t tiles 4..15
            3: lambda: cast_v(0, 1),
            4: lambda: (load_qt(1, 0), load_qt(1, 1), load_v(1)),
            6: lambda: cast_qt(1, 0),
            8: lambda: cast_qt(1, 1),
            10: lambda: (transpose_span(1, 0, 8),
                         transpose_span(1, 8, 20),
                         transpose_span(1, 20, 32)),
            12: lambda: cast_v(1, 0),
            14: lambda: cast_v(1, 1),
        }

        state = {}

        def qk_group(b, qb, g):
            q0 = qb * QB
            qt_tp = qt_tps[b]
            s_ps = qk_psum.tile([128, 1024], F32, tag="qk")
            num = num_pool.tile([128, 1024], BF16, tag="num")
            act_spans = []
            dve_ops = []   # (kind, lo, hi) kind: 'max' | 'stt'
            for j, c in enumerate((2 * g, 2 * g + 1)):
                i = c - 4 * qb
                lo = 128 * i if i > 0 else 0
                # rhs: q columns [q0+lo, q0+512) gathered from staged slots
                # (512-aligned within one chunk, so contiguous)
                ql = q0 + lo
                t0_, t1_ = ql // 128, (q0 + QB) // 128
                rhs = qt_tp[:, q_slot(t0_):q_slot(t0_) + (t1_ - t0_), :] \
                    .rearrange("p t q -> p (t q)")
                nc.tensor.matmul(
                    s_ps[:, j * 512 + lo:(j + 1) * 512],
                    lhsT=qt_tp[:, t_slot(c), :],
                    rhs=rhs,
                )
                if act_spans and act_spans[-1][1] == j * 512 + lo:
                    act_spans[-1] = (act_spans[-1][0], (j + 1) * 512)
                else:
                    act_spans.append((j * 512 + lo, (j + 1) * 512))
                if i >= 0:
                    dve_ops.append(("stt", j * 512 + lo, (j + 1) * 512))
                elif dve_ops and dve_ops[-1][0] == "max" \
                        and dve_ops[-1][2] == j * 512 + lo:
                    dve_ops[-1] = ("max", dve_ops[-1][1], (j + 1) * 512)
                else:
                    dve_ops.append(("max", j * 512 + lo, (j + 1) * 512))
            for lo, hi in act_spans:
                nc.scalar.activation(num[:, lo:hi], s_ps[:, lo:hi],
                                     mybir.ActivationFunctionType.Exp,
                                     scale=SCALE)
            for kind, lo, hi in dve_ops:
                if kind == "max":
                    nc.vector.tensor_scalar_max(num[:, lo:hi], num[:, lo:hi], 1.0)
                else:
                    nc.vector.scalar_tensor_tensor(
                        num[:, lo:hi], num[:, lo:hi], 1.0,
                        tri_pad01[:, 0:hi - lo],
                        op0=Alu.max, op1=Alu.mult)
            st = state.setdefault((b, qb), {"ob": None, "num": {}})
            if st["ob"] is None:
                st["ob"] = [ob_psum.tile([128, 2, 256], F32, tag="ob",
                                         name=f"ob_{b}_{qb}_{h}")
                            for h in range(2)]
            st["num"][g] = num

        def pv_group(b, qb, g):
            st = state[(b, qb)]
            num = st["num"].pop(g)
            v_aug = v_augs[b]
            for j, c in enumerate((2 * g, 2 * g + 1)):
                i = c - 4 * qb
                for sub in range(max(i, 0), 4):
                    ob = st["ob"][sub // 2]
                    nc.tensor.matmul(
                        ob[:, sub % 2, 0:129],
                        lhsT=num[:, j * 512 + sub * 128:j * 512 + (sub + 1) * 128],
                        rhs=v_aug[:, c, 0:129],
                        start=(c == 0 and sub % 2 == 0),
                        stop=(c == 4 * qb + sub),
                        skip_group_check=True,
                    )

        def finalize(b, qb):
            st = state.pop((b, qb))
            o_tile = fin_pool.tile([128, 4, 128], F32, tag="fin")
            for h in range(2):
                ob_sb = rec_pool.tile([128, 2, 129], F32, tag="rec")
                nc.vector.tensor_copy(ob_sb[:], st["ob"][h][:, :, 0:129])
                for s2 in range(2):
                    nc.gpsimd.normalize_recip(
                        o_tile[:, 2 * h + s2, :],
                        ob_sb[:, s2, 0:128],
                        ob_sb[:, s2, 128:129])
            nc.sync.dma_start(
                o_ext[b, qb * QB:(qb + 1) * QB, :]
                    .rearrange("(s p) d -> p s d", p=128),
                o_tile[:])

        pending = deque()

        def flush_one():
            b, qb, g = pending.popleft()
            pv_group(b, qb, g)
            if g == (4 * qb + 4) // 2 - 1:
                finalize(b, qb)

        for idx, it in enumerate(items):
            qk_group(*it)
            if idx in prep_at:
                prep_at[idx]()
            pending.append(it)
            if len(pending) > 2:
                flush_one()
        while pending:
            flush_one()

    nc.compile()
    return nc


_NC_CACHE = None


def _get_nc():
    global _NC_CACHE
    if _NC_CACHE is None:
        _NC_CACHE = build_attention_core()
    return _NC_CACHE


def kernel(Q: np.ndarray, T: np.ndarray, V: np.ndarray) -> np.ndarray:
    """Full-input entry point: shard over batch, run 8-core SPMD, gather."""
    from concourse.bass_utils import run_bass_kernel_spmd

    Q = np.ascontiguousarray(np.asarray(Q, dtype=np.float32))
    T = np.ascontiguousarray(np.asarray(T, dtype=np.float32))
    V = np.ascontiguousarray(np.asarray(V, dtype=np.float32))
    assert Q.shape == (B, S, D), Q.shape

    nc = _get_nc()
    in_maps = [
        {
            "Q": Q[i * B_LOC:(i + 1) * B_LOC],
            "T": T[i * B_LOC:(i + 1) * B_LOC],
            "V": V[i * B_LOC:(i + 1) * B_LOC],
        }
        for i in range(N_CORES)
    ]
    res = run_bass_kernel_spmd(nc, in_maps, core_ids=list(range(N_CORES)))
    return np.concatenate([res.results[i]["out"] for i in range(N_CORES)], axis=0)
